# revision 39
# baseline (speedup 1.0000x reference)
import os
import sys

if "/opt/trn_rl_repo" not in sys.path:
    sys.path.insert(0, "/opt/trn_rl_repo")

from contextlib import ExitStack

import ml_dtypes
import numpy as np

import concourse.bass as bass
import concourse.bacc as bacc
import concourse.tile as tile
from concourse import mybir
from concourse.bass_utils import run_bass_kernel_spmd

B, H, S, D = 2, 16, 2048, 64
NCORES = 8
PAIRS = (B * H) // NCORES
NT = S // 128
F32 = mybir.dt.float32
BF16 = mybir.dt.bfloat16
SCALE = 0.125
PV_N = int(os.environ.get("PV_N", "128"))
WARMUP = int(os.environ.get("WARMUP", "8"))


def build_nc():
    nc = bacc.Bacc(None)
    qT = nc.declare_dram_parameter("qT", [PAIRS, D, S], BF16, isOutput=False)
    kT = nc.declare_dram_parameter("kT", [PAIRS, D, S], BF16, isOutput=False)
    v = nc.declare_dram_parameter("v", [PAIRS, S, D], BF16, isOutput=False)
    out = nc.declare_dram_parameter("out", [PAIRS, S, D], F32, isOutput=True)

    with tile.TileContext(nc) as tc, ExitStack() as ctx:
        consts = ctx.enter_context(tc.tile_pool(name="consts", bufs=1))
        qtp = ctx.enter_context(tc.tile_pool(name="qt", bufs=2))
        ktp = ctx.enter_context(tc.tile_pool(name="kt", bufs=2))
        vpp = ctx.enter_context(tc.tile_pool(name="vp", bufs=2))
        ptp = ctx.enter_context(tc.tile_pool(name="pt", bufs=3))
        outp = ctx.enter_context(tc.tile_pool(name="outsb", bufs=2))
        smalls = ctx.enter_context(tc.tile_pool(name="smalls", bufs=4))
        ps_scores = ctx.enter_context(
            tc.tile_pool(name="ps_scores", bufs=2, space="PSUM")
        )
        ps_acc = ctx.enter_context(tc.tile_pool(name="ps_acc", bufs=1, space="PSUM"))

        tri01 = consts.tile([128, 128], BF16)
        nc.gpsimd.memset(tri01, 1.0)
        nc.gpsimd.affine_select(
            out=tri01,
            in_=tri01,
            compare_op=mybir.AluOpType.is_ge,
            fill=0.0,
            base=0,
            pattern=[[1, 128]],
            channel_multiplier=-1,
        )

        t01 = tri01[:, :]
        tri_rep = bass.AP(
            tensor=t01.tensor,
            offset=t01.offset,
            ap=[t01.ap[0], [0, 4], t01.ap[1]],
        )
        if WARMUP:
            wq = ps_scores.tile([128, 1024], F32, tag="scores")
            for _ in range(WARMUP):
                nc.tensor.matmul(wq[:, 0:512], tri01, tri_rep, start=True, stop=True)
        tbl = smalls.tile([128, 1], F32, tag="rec", name="tbl")
        nc.scalar.activation(tbl, tri01[:, 0:1], mybir.ActivationFunctionType.Exp)

        def load_pair(p):
            qt = qtp.tile([128, S], BF16, tag="qt")
            kt = ktp.tile([128, S], BF16, tag="kt")
            vp_t = vpp.tile([128, NT, 128], BF16, tag="vp")
            hq, hk = 1024, 256
            for r0 in (0, D):
                nc.sync.dma_start(out=qt[r0 : r0 + D, 0:hq], in_=qT[p][:, 0:hq])
                nc.sync.dma_start(out=kt[r0 : r0 + D, 0:hk], in_=kT[p][:, 0:hk])
            for r0 in (0, D):
                nc.sync.dma_start(out=qt[r0 : r0 + D, hq:], in_=qT[p][:, hq:])
                nc.sync.dma_start(out=kt[r0 : r0 + D, hk:], in_=kT[p][:, hk:])
            nc.sync.dma_start(
                out=vp_t[:, :, 0:D],
                in_=v[p].rearrange("(t pp) d -> pp t d", pp=128),
            )
            nc.vector.memset(vp_t[:, :, D : D + 1], 1.0)
            nc.vector.memset(vp_t[:, :, D + 1 :], 0.0)
            return {
                "qt": qt,
                "kt": kt,
                "vp": vp_t,
                "acc": None,
                "out_r": out[p].rearrange("(t pp) d -> pp t d", pp=128),
            }

        def pcol(j, c):
            return 1024 * (c // 512) + (512 if j % 2 else 0) + (c % 512)

        def emit_seg_pair(st, ja, jb, ptab, si):
            qt, kt = st["qt"], st["kt"]
            wa = min(512, S - ja * 128 - 512 * si)
            wb = min(512, S - jb * 128 - 512 * si)
            ps = ps_scores.tile([128, 1024], F32, tag="scores")
            for r0, j, w in ((0, ja, wa), (D, jb, wb)):
                q0 = j * 128
                off = 512 * si
                nc.tensor.matmul(
                    ps[:, r0 * 8 : r0 * 8 + w],
                    kt[r0 : r0 + D, q0 : q0 + 128],
                    qt[r0 : r0 + D, q0 + off : q0 + off + w],
                    start=True,
                    stop=True,
                    tile_position=(r0, 0),
                )
            nc.scalar.activation(
                ptab[:, 1024 * si : 1024 * si + 512 + wb],
                ps[:, 0 : 512 + wb],
                mybir.ActivationFunctionType.Exp,
                scale=SCALE,
            )
            if si == 0:
                nc.vector.tensor_mul(ptab[:, 0:128], ptab[:, 0:128], tri01)
                nc.vector.tensor_mul(ptab[:, 512:640], ptab[:, 512:640], tri01)

        def pv_mms(st, j, ptab):
            acc = st["acc"]
            for i in range(j, NT):
                c = pcol(j, (i - j) * 128)
                yield (
                    acc[:, i, 0:PV_N],
                    ptab[:, c : c + 128],
                    st["vp"][:, j, 0:PV_N],
                    j == 0 and i % 4 == 0,
                    j == i,
                )

        def emit_pv_mm(mm):
            o, l, r, st_, sp = mm
            nc.tensor.matmul(o, l, r, start=st_, stop=sp)

        def emit_finish(st, g):
            acc = st["acc"]
            g0 = 4 * g
            accs = outp.tile([128, 4, D + 1], F32, tag="accs")
            nc.vector.tensor_copy(accs, acc[:, g0 : g0 + 4, 0 : D + 1])
            rec4 = smalls.tile([128, 4], F32, tag="rec")
            nc.vector.reciprocal(rec4, accs[:, :, D])
            osb = outp.tile([128, 4, D], F32, tag="osb")
            r4 = rec4[:, :]
            rec_bcast = bass.AP(
                tensor=r4.tensor,
                offset=r4.offset,
                ap=[r4.ap[0], r4.ap[1], [0, D]],
            )
            nc.vector.tensor_mul(osb, accs[:, :, 0:D], rec_bcast)
            nc.sync.dma_start(out=st["out_r"][:, g0 : g0 + 4, :], in_=osb)

        states = [None] * PAIRS
        states[0] = load_pair(0)
        pending = []

        def flush_one():
            fst, oa, ob, opab = pending.pop(0)
            pv = list(pv_mms(fst, oa, opab)) + list(pv_mms(fst, ob, opab))
            fin = ob // 4 if ob % 4 == 3 else None
            return fst, pv, fin

        for p in range(PAIRS):
            st = states[p]
            if p + 1 < PAIRS:
                states[p + 1] = load_pair(p + 1)
            st["acc"] = ps_acc.tile([128, NT, 128], F32, tag="acc", name="acc_t")
            for jp in range(0, NT, 2):
                ja, jb = jp, jp + 1
                ptab = ptp.tile([128, 4096], BF16, tag="pt")
                nseg = (S - ja * 128 + 511) // 512
                pv, fin, fst = [], None, None
                depth = 1 if (p == PAIRS - 1 and jp >= NT - 4) else 2
                if len(pending) >= depth:
                    fst, pv, fin = flush_one()
                per_slot = (len(pv) + nseg - 1) // nseg if pv else 0
                k = 0
                for si in range(nseg):
                    emit_seg_pair(st, ja, jb, ptab, si)
                    take = pv[k : k + per_slot] if si < nseg - 1 else pv[k:]
                    for mm in take:
                        emit_pv_mm(mm)
                    k += len(take)
                if fin is not None:
                    emit_finish(fst, fin)
                pending.append((st, ja, jb, ptab))
        while pending:
            fst, pv, fin = flush_one()
            for mm in pv:
                emit_pv_mm(mm)
            if fin is not None:
                emit_finish(fst, fin)
    nc.compile()
    return nc


_nc_cache = None


def _get_nc():
    global _nc_cache
    if _nc_cache is None:
        _nc_cache = build_nc()
    return _nc_cache


def kernel(q, k, v, mask):
    nc = _get_nc()
    bf = ml_dtypes.bfloat16
    qf = np.asarray(q, dtype=np.float32).reshape(B * H, S, D)
    kf = np.asarray(k, dtype=np.float32).reshape(B * H, S, D)
    vf = np.ascontiguousarray(
        np.asarray(v, dtype=np.float32).reshape(B * H, S, D).astype(bf)
    )
    qTf = np.ascontiguousarray(qf.transpose(0, 2, 1).astype(bf))
    kTf = np.ascontiguousarray(kf.transpose(0, 2, 1).astype(bf))
    in_maps = [
        {
            "qT": qTf[i * PAIRS : (i + 1) * PAIRS],
            "kT": kTf[i * PAIRS : (i + 1) * PAIRS],
            "v": vf[i * PAIRS : (i + 1) * PAIRS],
        }
        for i in range(NCORES)
    ]
    res = run_bass_kernel_spmd(nc, in_maps, core_ids=list(range(NCORES)))
    o = np.concatenate([res.results[i]["out"] for i in range(NCORES)], axis=0)
    return o.reshape(B, H, S, D)
